# revision 1
# baseline (speedup 1.0000x reference)
"""GCNConv-style GNN layer on 8 Trainium2 NeuronCores (Bass/Tile).

Reference computation (B=8, N=4096, C=128, E=131072):
    adj  = symmetric 0/1 adjacency from edge_index, zero diagonal
    h    = x @ W0 + b0
    agg  = adj @ h            (per batch)
    out  = (cat[x, agg] @ W1 + b1) @ W2 + b2
    out  = gelu(out) @ Wo + bo
    ret  = x + out

Algebraic refactor (all linear maps before the single GELU compose; fold
them on the host at O(C^2) cost):
    W12  = W1 @ W2                  [2C, C]
    Wx   = W12[:C]                  x-path weight
    Wa   = W0 @ W12[C:]             agg-path weight applied to s = adj @ x
    b0a  = b0 @ W12[C:]
    b12  = b1 @ W2 + b2
    pre  = x @ Wx + (adj @ x) @ Wa + deg (x) b0a + b12
    ret  = x + gelu(pre) @ Wo + bo
where deg = adj.sum(1).

Sharding: 2 node-groups x 4 batch-groups (each core: 2048 nodes, 2
batches).  This keeps the per-core adjacency slice at 8MB fp8 (a bf16
slice starves the PE behind the ~240GB/s per-stream DMA rate) and gives
each stationary operand of the s = adj @ x contraction 4 moving matmuls
of reuse.

The graded variant is "dr2": the s-stage runs in fp8 e4m3 with
perf_mode=DoubleRow, packing two 128-row k-chunks per matmul.  Measured
on this hardware a DR matmul streams at the bf16 element rate (no fp8
double-pump through the walrus-mandated slab-split ifmap AP), so the
win over bf16 is halved instruction count (each matmul pays its serial
~213ns weight load) and halved adjacency DMA.  The fused MLP stays bf16
with fp32 PSUM accumulation; the residual uses bf16 x plus an exact bo
bias on the ACT engine (adds ~2^-9|x| error, net 1.22e-2 vs the 2e-2
gate -- verified against the reference on CPU and on hardware).  Unused
per-matmul PE semaphore increments are stripped in the single-shot
build.  Other variants ("bf16", "fp8", "dr", "dr_pa", "swi") are kept
for A/B experiments driven by test.py.
"""

import numpy as np
import ml_dtypes

import bass_rust
import concourse.bass as bass
import concourse.mybir as mybir
import concourse.tile as tile
from concourse.bass_utils import run_bass_kernel_spmd

B, N, C, E = 8, 4096, 128, 131072
NCORES = 8
NG = 2                 # node groups
BG = 4                 # batch groups
NS2 = N // NG          # 2048 nodes per core
BP = B // BG           # 2 batches per core
COLS = BP * C          # 256 xr columns (b-major, c-minor)
RCOLS = BP * NS2       # 4096 output columns (b-major, node-minor)
KC = N // 128          # 32 k-chunks over the contraction dim
KP = KC // 2           # 16 DoubleRow k-pairs
NJ = NS2 // 512        # 4 node chunks of 512 per core

F32 = mybir.dt.float32
BF16 = mybir.dt.bfloat16
F8 = mybir.dt.float8e4
BF16_NP = ml_dtypes.bfloat16
F8_NP = ml_dtypes.float8_e4m3

VARIANT = "dr2"        # "bf16" | "fp8" | "dr" | "dr2" | "dr_pa" | "swi"


def _split_multiwaits(nc, max_waits=1):
    """Walrus (CoreV3) refuses instructions with more than one sync wait.
    Tile's tail drain can carry several; hoist the extras onto preceding
    single-wait EventSemaphore instructions on the same engine."""
    for blk in nc.m.functions[0].blocks:
        new_list = []
        for ins in blk.instructions:
            si = ins.sync_info
            if si is not None and si.on_wait and len(si.on_wait) > max_waits:
                waits = list(si.on_wait)
                extra, keep = waits[:-max_waits], waits[-max_waits:]
                for i, w in enumerate(extra):
                    ev = mybir.InstEventSemaphore(
                        name=f"{ins.name}_wsplit{i}",
                        engine=ins.engine,
                        ins=[],
                        outs=[],
                        sync_info=bass_rust.SyncInfo(on_wait=[w], on_update=[]),
                    )
                    new_list.append(ev)
                si.on_wait = keep
            new_list.append(ins)
        blk.instructions[:] = new_list


def _dedup_ldweights(nc):
    """Bass legalization emits one InstLdweights per InstMatmult even when
    consecutive matmuls share the identical stationary operand.  Drop
    loads whose weights AP matches the PE array's current contents.  Only
    loads with no sync waits/updates are dropped (the first load of each
    reuse group carries the DMA wait and is kept)."""

    def sig(ins):
        w = ins.ins[0]
        return (
            w.offset,
            tuple(tuple(p) for p in w.ap),
            str(w.dtype),
            getattr(w, "memref", None),
            getattr(w, "memsetref", None),
            str(ins.perf_mode),
            bool(ins.is_transpose),
        )

    for blk in nc.m.functions[0].blocks:
        last = None
        kept = []
        for ins in blk.instructions:
            tn = type(ins).__name__
            if tn == "InstLdweights":
                si = ins.sync_info
                clean = si is None or (not si.on_wait and not si.on_update)
                s = sig(ins)
                if clean and s == last:
                    continue
                last = s
            kept.append(ins)
        blk.instructions[:] = kept


def _fuse_ldweights(nc):
    """Remove the legalization-split InstLdweights and mark each
    InstMatmult self-loading, migrating the load's sync waits onto the
    matmul.  Required for walrus --enable-ldw-opt=true, which rejects
    explicit InstLdweights but then optimizes the weight loads itself."""
    for blk in nc.m.functions[0].blocks:
        kept = []
        pending = []
        for ins in blk.instructions:
            tn = type(ins).__name__
            if tn == "InstLdweights":
                si = ins.sync_info
                if si is not None:
                    pending.extend(si.on_wait or [])
                    assert not si.on_update, "ldweights with updates unsupported"
                continue
            if tn == "InstMatmult":
                ins.ldweights = True
                if pending:
                    si = ins.sync_info
                    if si is None:
                        ins.sync_info = bass_rust.SyncInfo(
                            on_wait=list(pending), on_update=[])
                    else:
                        si.on_wait = list(pending) + list(si.on_wait or [])
                    pending = []
            kept.append(ins)
        assert not pending, "dangling ldweights waits"
        blk.instructions[:] = kept


def _strip_pe_semupds(nc, relocate=False):
    """Every InstMatmult increments the PE semaphore, but only ~25 of the
    160 values are ever awaited.  Strip the unneeded increments (engine
    EVT_SEM writes cost serial issue time) and rewrite every wait on the
    PE semaphore (in all blocks) to the new, sparser counting.
    relocate=True re-emits the stripped increments as InstEventSemaphore
    ticks after the last matmul, preserving the per-iteration total that
    the Tile hardware-loop boundary requires (mandatory for niter>1)."""
    blocks = nc.m.functions[0].blocks
    pe_incs = []  # (block, instruction, update) in program order
    for blk in blocks:
        for ins in blk.instructions:
            si = ins.sync_info
            if si is None:
                continue
            for u in (si.on_update or []):
                if u.ant_name.startswith("PE_"):
                    pe_incs.append((blk, ins, u))
    if len(pe_incs) < 8:
        return
    total = len(pe_incs)  # walrus enforces update_value==1
    needed_vals = set()
    for blk in blocks:
        for ins in blk.instructions:
            si = ins.sync_info
            if si is None:
                continue
            for w in (si.on_wait or []):
                if w.ant_name.startswith("PE_") and w.wait_value is not None:
                    needed_vals.add(w.wait_value)
    needed_idx = set()
    for v in needed_vals:
        assert v <= total, f"PE wait {v} beyond total {total}"
        needed_idx.add(v - 1)  # inc #v (1-based) satisfies value v
    needed_idx.add(total - 1)  # keep the final one
    new_cum = []
    kept = 0
    for i in range(total):
        if i in needed_idx:
            kept += 1
        new_cum.append(kept)
    n_stripped = total - kept
    if not relocate:
        val_map = {v: new_cum[v - 1] for v in needed_vals}
        for blk in blocks:
            for ins in blk.instructions:
                si = ins.sync_info
                if si is None:
                    continue
                for w in (si.on_wait or []):
                    if w.ant_name.startswith("PE_") and w.wait_value is not None:
                        w.wait_value = val_map[w.wait_value]
    else:
        # kept incs count first; the relocated tail ticks bring the total
        # back to `total`, so only intermediate waits need remapping
        val_map = {v: new_cum[v - 1] for v in needed_vals}
        for blk in blocks:
            for ins in blk.instructions:
                si = ins.sync_info
                if si is None:
                    continue
                for w in (si.on_wait or []):
                    if (w.ant_name.startswith("PE_")
                            and w.wait_value is not None
                            and w.wait_value < total):
                        w.wait_value = val_map[w.wait_value]
    for i, (blk, ins, u) in enumerate(pe_incs):
        if i not in needed_idx:
            ins.sync_info.on_update = [x for x in ins.sync_info.on_update if x is not u]
    if relocate and n_stripped:
        lb, last_ins, lu = pe_incs[-1]
        pos = lb.instructions.index(last_ins) + 1
        ticks = []
        for i in range(n_stripped):
            ticks.append(mybir.InstEventSemaphore(
                name=f"{last_ins.name}_semtick{i}",
                engine=last_ins.engine,
                ins=[],
                outs=[],
                sync_info=bass_rust.SyncInfo(
                    on_wait=[],
                    on_update=[bass_rust.SyncUpdate(
                        sync_type=lu.sync_type,
                        id=lu.id,
                        ant_name=lu.ant_name,
                        update_mode=lu.update_mode,
                        update_value=1,
                        update_reg=None,
                    )],
                ),
            ))
        lb.instructions[pos:pos] = ticks


_LDWOPT_PATCHED = []


def _patch_ldwopt():
    """Flip walrus --enable-ldw-opt to true for subsequent compiles."""
    if _LDWOPT_PATCHED:
        return
    import concourse.bass_utils as _bu

    _orig = _bu.run_command

    def patched(cmd, **kw):
        cmd = [
            c.replace("--enable-ldw-opt=false", "--enable-ldw-opt=true")
            if isinstance(c, str) else c
            for c in cmd
        ]
        return _orig(cmd, **kw)

    _bu.run_command = patched
    _LDWOPT_PATCHED.append(True)


def build_bass(niter=1, stage="full", variant=None, dedup=False, ldwopt=False,
               strip=True):
    """Build the SPMD program.  niter>1 wraps the body in a Tile For_i
    loop -- used only for hardware timing (amortizes axon dispatch
    overhead); the graded kernel uses niter=1.
    stage: "full" | "s_only" (timing ablation)."""
    variant = variant or VARIANT
    sdt = BF16 if variant == "bf16" else F8
    nc = bass.Bass()

    if variant in ("dr2", "dr3", "dr4"):
        # partition-major xr (one contiguous DMA), no f32 residual input
        xr_d = nc.dram_tensor("xr", [128, KC * COLS], F8, kind="ExternalInput")
        if variant == "dr4":
            # partition-major pair-merged adjacency: 16 DMAs, 4KB lines
            adjT_d = nc.dram_tensor("adjT", [128, KC * NS2], F8, kind="ExternalInput")
        else:
            adjT_d = nc.dram_tensor("adjT", [N, NS2], F8, kind="ExternalInput")
    elif variant == "dr_pa":
        # pair-adjacent fp8 layout: dram matches SBUF exactly, partition-major
        xr_d = nc.dram_tensor("xr", [128, KP * BP * 128 * 2], F8, kind="ExternalInput")
        adjT_d = nc.dram_tensor("adjT", [128, KP * NS2 * 2], F8, kind="ExternalInput")
    elif variant == "swi":
        # sw-interleaved stationary (contiguous per-(t,b) 256-col weight),
        # adjacency chunk layout as in "dr"
        xr_d = nc.dram_tensor("xr", [128, KP * BP * 256], F8, kind="ExternalInput")
        adjT_d = nc.dram_tensor("adjT", [N, NS2], F8, kind="ExternalInput")
    else:
        xr_d = nc.dram_tensor("xr", [N, COLS], sdt, kind="ExternalInput")
        adjT_d = nc.dram_tensor("adjT", [N, NS2], sdt, kind="ExternalInput")
    xt_bf_d = nc.dram_tensor("xt_bf", [C, RCOLS], BF16, kind="ExternalInput")
    if variant in ("dr2", "dr3", "dr4"):
        bo_d = nc.dram_tensor("bo", [C, 1], F32, kind="ExternalInput")
    else:
        xtbo_d = nc.dram_tensor("xtbo", [C, RCOLS], F32, kind="ExternalInput")
    deg_d = nc.dram_tensor("deg", [1, NS2], BF16, kind="ExternalInput")
    b0a_d = nc.dram_tensor("b0a", [1, C], BF16, kind="ExternalInput")
    wx_d = nc.dram_tensor("wx", [C, C], BF16, kind="ExternalInput")
    wa_d = nc.dram_tensor("wa", [C, C], BF16, kind="ExternalInput")
    wo_d = nc.dram_tensor("wo", [C, C], BF16, kind="ExternalInput")
    b12_d = nc.dram_tensor("b12", [C, 1], F32, kind="ExternalInput")
    out_d = nc.dram_tensor("out", [C, RCOLS], F32, kind="ExternalOutput")

    with tile.TileContext(nc) as tc:
        with (
            tc.tile_pool(name="const", bufs=1) as const,
            tc.tile_pool(name="big", bufs=1) as big,
        ):

            def body(_iv=0):
                # ---- resident inputs -------------------------------------
                if variant == "dr_pa":
                    xr_sb = big.tile([128, KP, BP * 128, 2], F8)
                    adjT_sb = big.tile([128, KP, NS2, 2], F8)
                elif variant == "swi":
                    xr_sb = big.tile([128, KP, BP, 256], F8)
                    adjT_sb = big.tile([128, KC, NS2], F8)
                else:
                    xr_sb = big.tile([128, KC, COLS], sdt)
                    adjT_sb = big.tile([128, KC, NS2], sdt)
                xt_bf_sb = big.tile([C, RCOLS], BF16)
                if variant in ("dr2", "dr3", "dr4"):
                    bo_sb = const.tile([C, 1], F32)
                else:
                    xtbo_sb = big.tile([C, RCOLS], F32)
                wx_sb = const.tile([C, C], BF16)
                wa_sb = const.tile([C, C], BF16)
                wo_sb = const.tile([C, C], BF16)
                b12_sb = const.tile([C, 1], F32)
                if variant == "dr3":
                    # deg/b0a replicated at partitions 0/32/64/96 so the 8
                    # rank-1 matmuls can pack 4-concurrent into distinct PE
                    # row groups
                    deg_sb = const.tile([128, NS2], BF16)
                    b0a_sb = const.tile([128, C], BF16)
                else:
                    deg_sb = const.tile([1, NS2], BF16)
                    b0a_sb = const.tile([1, C], BF16)
                if variant == "dr_pa":
                    for t in range(KP):
                        nc.sync.dma_start(out=adjT_sb[:, t, :, :],
                                          in_=adjT_d[:, t * NS2 * 2:(t + 1) * NS2 * 2])
                        nc.sync.dma_start(out=xr_sb[:, t, :, :],
                                          in_=xr_d[:, t * BP * 256:(t + 1) * BP * 256])
                elif variant == "swi":
                    for k in range(KC):
                        nc.sync.dma_start(out=adjT_sb[:, k, :], in_=adjT_d[k * 128:(k + 1) * 128, :])
                    nc.sync.dma_start(out=xr_sb[:, :, :, :], in_=xr_d[:, :])
                elif variant == "dr2":
                    nc.sync.dma_start(out=xr_sb[:, :, :], in_=xr_d[:, :])
                    for k in range(KC):
                        nc.sync.dma_start(out=adjT_sb[:, k, :], in_=adjT_d[k * 128:(k + 1) * 128, :])
                elif variant == "dr4":
                    nc.sync.dma_start(out=xr_sb[:, :, :], in_=xr_d[:, :])
                    for t in range(KP):
                        nc.sync.dma_start(out=adjT_sb[:, 2 * t:2 * t + 2, :],
                                          in_=adjT_d[:, t * 2 * NS2:(t + 1) * 2 * NS2])
                elif variant == "dr3":
                    # first 4 chunks land fast so the PE starts sooner; the
                    # rest follows as one big transfer
                    nc.sync.dma_start(out=xr_sb[:, 0:4, :], in_=xr_d[:, 0:4 * COLS])
                    nc.sync.dma_start(out=xr_sb[:, 4:KC, :], in_=xr_d[:, 4 * COLS:])
                    for k in range(KC):
                        nc.sync.dma_start(out=adjT_sb[:, k, :], in_=adjT_d[k * 128:(k + 1) * 128, :])
                else:
                    for k in range(KC):
                        nc.sync.dma_start(out=adjT_sb[:, k, :], in_=adjT_d[k * 128:(k + 1) * 128, :])
                        nc.sync.dma_start(out=xr_sb[:, k, :], in_=xr_d[k * 128:(k + 1) * 128, :])
                nc.sync.dma_start(out=wx_sb[:], in_=wx_d[:])
                nc.sync.dma_start(out=wa_sb[:], in_=wa_d[:])
                nc.sync.dma_start(out=wo_sb[:], in_=wo_d[:])
                nc.sync.dma_start(out=b12_sb[:], in_=b12_d[:])
                if variant == "dr3":
                    nc.sync.dma_start(out=deg_sb[0:1, :], in_=deg_d[:])
                    nc.sync.dma_start(out=b0a_sb[0:1, :], in_=b0a_d[:])
                    for q in (32, 64, 96):
                        nc.vector.tensor_copy(out=deg_sb[q:q + 1, :], in_=deg_sb[0:1, :])
                        nc.vector.tensor_copy(out=b0a_sb[q:q + 1, :], in_=b0a_sb[0:1, :])
                else:
                    nc.sync.dma_start(out=deg_sb[:], in_=deg_d[:])
                    nc.sync.dma_start(out=b0a_sb[:], in_=b0a_d[:])
                nc.sync.dma_start(out=xt_bf_sb[:], in_=xt_bf_d[:])
                if variant in ("dr2", "dr3", "dr4"):
                    nc.sync.dma_start(out=bo_sb[:], in_=bo_d[:])
                else:
                    nc.sync.dma_start(out=xtbo_sb[:], in_=xtbo_d[:])

                # ---- s = adj @ x, transposed: accumulator (b, j) holds
                # sT[c, node-chunk j] for local batch b.  Stationary =
                # xr k-chunk for batch b, reused across the 4 node-chunk
                # moving operands.  k outer keeps the PE chasing the DMA
                # stream from the first chunk. ----
                sT_sb = big.tile([C, RCOLS], BF16)
                gelu_sb = big.tile([C, RCOLS], BF16)
                res_sb = big.tile([C, RCOLS], F32)
                chunks = [(b, j) for b in range(BP) for j in range(NJ)]

                with tc.tile_pool(name="psum", bufs=8, space="PSUM") as psum:
                    ps = {
                        (b, j): psum.tile([128, 512], F32, tag="ps", name=f"sT_{b}_{j}")
                        for (b, j) in chunks
                    }
                    if variant in ("dr", "dr2", "dr3", "dr4"):
                        for t in range(KP):
                            for b in range(BP):
                                lhsT = xr_sb[:, 2 * t:2 * t + 2, b * 128:(b + 1) * 128]
                                for j in range(NJ):
                                    nc.tensor.matmul(
                                        ps[(b, j)],
                                        lhsT,
                                        adjT_sb[:, 2 * t:2 * t + 2, j * 512:(j + 1) * 512],
                                        start=(t == 0),
                                        stop=(t == KP - 1),
                                        perf_mode=mybir.MatmulPerfMode.DoubleRow,
                                    )
                    elif variant == "dr_pa":
                        for t in range(KP):
                            for b in range(BP):
                                lhsT = xr_sb[:, t, b * 128:(b + 1) * 128, :]
                                for j in range(NJ):
                                    nc.tensor.matmul(
                                        ps[(b, j)],
                                        lhsT,
                                        adjT_sb[:, t, j * 512:(j + 1) * 512, :],
                                        start=(t == 0),
                                        stop=(t == KP - 1),
                                        perf_mode=mybir.MatmulPerfMode.DoubleRow,
                                    )
                    elif variant == "swi":
                        for t in range(KP):
                            for b in range(BP):
                                lhsT = xr_sb[:, t, b, :]
                                for j in range(NJ):
                                    nc.tensor.matmul(
                                        ps[(b, j)],
                                        lhsT,
                                        adjT_sb[:, 2 * t:2 * t + 2, j * 512:(j + 1) * 512],
                                        start=(t == 0),
                                        stop=(t == KP - 1),
                                        perf_mode=mybir.MatmulPerfMode.DoubleRowSwInterleave,
                                    )
                    else:
                        for t in range(KC):
                            for b in range(BP):
                                lhsT = xr_sb[:, t, b * 128:(b + 1) * 128]
                                for j in range(NJ):
                                    nc.tensor.matmul(
                                        ps[(b, j)],
                                        lhsT,
                                        adjT_sb[:, t, j * 512:(j + 1) * 512],
                                        start=(t == 0),
                                        stop=(t == KC - 1),
                                    )

                    # evacs on DVE begin as soon as each bank's stop MM
                    # retires (bank (0,0) frees 7 MMs before s ends)
                    for (b, j) in chunks:
                        cs = slice(b * NS2 + j * 512, b * NS2 + (j + 1) * 512)
                        nc.vector.tensor_copy(out=sT_sb[:, cs], in_=ps[(b, j)])

                    if stage != "full":
                        nc.sync.dma_start(
                            out=out_d[:, 0:RCOLS // 2],
                            in_=sT_sb.bitcast(F32)[:, 0:RCOLS // 2],
                        )
                        return

                    # ---- fused MLP, one 8-wide wave: each weight is
                    # loaded once and streamed over all 8 chunks.  pre
                    # reuses the s banks (freed by the DVE evacs), po
                    # reuses them again (freed by the ACT gelu reads).
                    def cslice(b, j):
                        return slice(b * NS2 + j * 512, b * NS2 + (j + 1) * 512)

                    pre = {
                        (b, j): psum.tile([128, 512], F32, tag="ps", name=f"pre_{b}_{j}")
                        for (b, j) in chunks
                    }
                    for (b, j) in chunks:
                        nc.tensor.matmul(pre[(b, j)], wx_sb[:], xt_bf_sb[:, cslice(b, j)],
                                         start=True, stop=False)
                    for (b, j) in chunks:
                        nc.tensor.matmul(pre[(b, j)], wa_sb[:], sT_sb[:, cslice(b, j)],
                                         start=False, stop=False)
                    if variant == "dr3":
                        # 4-way row-group packing: K=1 matmuls at partition
                        # bases 0/32/64/96 run concurrently in the PE array
                        for i, (b, j) in enumerate(chunks):
                            q = (i % 4) * 32
                            nc.tensor.matmul(
                                pre[(b, j)],
                                b0a_sb[q:q + 1, :],
                                deg_sb[q:q + 1, j * 512:(j + 1) * 512],
                                start=False, stop=True,
                                tile_position=(q, 0),
                            )
                    else:
                        for (b, j) in chunks:
                            nc.tensor.matmul(pre[(b, j)], b0a_sb[:], deg_sb[:, j * 512:(j + 1) * 512],
                                             start=False, stop=True)
                    for (b, j) in chunks:
                        nc.scalar.activation(
                            out=gelu_sb[:, cslice(b, j)], in_=pre[(b, j)],
                            func=mybir.ActivationFunctionType.Gelu,
                            bias=b12_sb[:, 0:1], scale=1.0,
                        )
                    po = {
                        (b, j): psum.tile([128, 512], F32, tag="ps", name=f"out_{b}_{j}")
                        for (b, j) in chunks
                    }
                    for (b, j) in chunks:
                        nc.tensor.matmul(po[(b, j)], wo_sb[:], gelu_sb[:, cslice(b, j)],
                                         start=True, stop=True)
                    for (b, j) in chunks:
                        cs = cslice(b, j)
                        if variant in ("dr2", "dr3", "dr4"):
                            # ACT evacuates po with the bo bias; DVE adds the
                            # bf16 residual (error ~2^-9 |x|, well in budget)
                            nc.scalar.activation(
                                out=res_sb[:, cs], in_=po[(b, j)],
                                func=mybir.ActivationFunctionType.Identity,
                                bias=bo_sb[:, 0:1], scale=1.0,
                            )
                            nc.vector.tensor_add(out=res_sb[:, cs], in0=res_sb[:, cs],
                                                 in1=xt_bf_sb[:, cs])
                        else:
                            nc.vector.tensor_add(out=res_sb[:, cs], in0=po[(b, j)],
                                                 in1=xtbo_sb[:, cs])
                        nc.sync.dma_start(out=out_d[:, cs], in_=res_sb[:, cs])

            if niter == 1:
                body()
            else:
                with tc.For_i(0, niter, 1, hint_engines=(mybir.EngineType.PE,)):
                    body()

    if ldwopt:
        _fuse_ldweights(nc)
        _patch_ldwopt()
    elif dedup:
        _dedup_ldweights(nc)
    if variant in ("dr2", "dr3", "dr4") and strip and niter == 1:
        # the sparser counting is incompatible with Tile's For_i iteration
        # accounting (hangs for niter>1); the graded single-shot build is safe
        _strip_pe_semupds(nc)
    _split_multiwaits(nc)
    return nc


def host_prep(x, edge_index, W0, b0, W1, b1, W2, b2, Wo, bo, variant=None):
    """Fold weights, build the dense adjacency, lay out per-core inputs."""
    variant = variant or VARIANT
    x = np.asarray(x, np.float32)
    ei = np.asarray(edge_index, np.int64)
    W0, b0, W1, b1, W2, b2, Wo, bo = (
        np.asarray(a, np.float32) for a in (W0, b0, W1, b1, W2, b2, Wo, bo)
    )

    # dense symmetric adjacency with set-semantics dedup, zero diagonal
    k1 = ei[0] * N + ei[1]
    k2 = ei[1] * N + ei[0]
    keys = np.unique(np.concatenate([k1, k2]))
    rows = keys // N
    cols = keys % N
    off_diag = rows != cols
    keys, rows = keys[off_diag], rows[off_diag]
    if variant == "bf16":
        adj = np.zeros(N * N, np.uint16)
        adj[keys] = 0x3F80  # bf16 1.0 bit pattern
        adj = adj.reshape(N, N).view(BF16_NP)
        s_np = BF16_NP
    else:
        adj = np.zeros(N * N, np.uint8)
        adj[keys] = 0x38  # fp8 e4m3 1.0 bit pattern
        adj = adj.reshape(N, N).view(F8_NP)
        s_np = F8_NP
    deg = np.bincount(rows, minlength=N).astype(np.float32)

    # folded weights
    W12 = W1 @ W2                      # [2C, C]
    Wx = W12[:C]
    W12a = W12[C:]
    Wa = W0 @ W12a
    b0a = b0 @ W12a                    # [C]
    b12 = (b1 @ W2 + b2).reshape(C, 1)

    xn = x.transpose(1, 0, 2)                                  # [N, B, C]
    xt = x.transpose(2, 0, 1)                                  # [C, B, N] f32

    in_maps = []
    for c in range(NCORES):
        g = c // BG                    # node group
        p = c % BG                     # batch pair
        rs = slice(g * NS2, (g + 1) * NS2)
        bs = slice(p * BP, (p + 1) * BP)
        xt_c = np.ascontiguousarray(xt[:, bs, rs]).reshape(C, RCOLS)
        if variant == "dr_pa":
            # pair-adjacent fp8: byte o of the 16-bit fetch at (p, col) is
            # contraction row 256t + 128o + p
            a = np.ascontiguousarray(adj[:, rs]).reshape(KP, 2, 128, NS2)
            adjT_c = np.ascontiguousarray(a.transpose(2, 0, 3, 1)).reshape(128, -1)
            xv = xn[:, bs, :].reshape(KP, 2, 128, BP, C).astype(s_np)
            xr_c = np.ascontiguousarray(xv.transpose(2, 0, 3, 4, 1)).reshape(128, -1)
        elif variant == "swi":
            # sw-interleaved stationary: per (t, b) the 256 weight columns are
            # [A127, B127, A126, B126, ...] (slabs interleaved, columns
            # reversed); A = k-chunk 2t, B = k-chunk 2t+1
            adjT_c = np.ascontiguousarray(adj[:, rs])
            xv = xn[:, bs, :].reshape(KP, 2, 128, BP, C).astype(s_np)
            rev = xv[:, :, :, :, ::-1]
            xr_c = np.ascontiguousarray(rev.transpose(2, 0, 3, 4, 1)).reshape(128, -1)
        elif variant == "dr4":
            a = np.ascontiguousarray(adj[:, rs]).reshape(KP, 2, 128, NS2)
            adjT_c = np.ascontiguousarray(a.transpose(2, 0, 1, 3)).reshape(128, -1)
            xv = xn[:, bs, :].reshape(KC, 128, COLS).astype(s_np)
            xr_c = np.ascontiguousarray(xv.transpose(1, 0, 2)).reshape(128, -1)
        elif variant in ("dr2", "dr3"):
            adjT_c = np.ascontiguousarray(adj[:, rs])
            xv = xn[:, bs, :].reshape(KC, 128, COLS).astype(s_np)
            xr_c = np.ascontiguousarray(xv.transpose(1, 0, 2)).reshape(128, -1)
        else:
            adjT_c = np.ascontiguousarray(adj[:, rs])
            xr_c = np.ascontiguousarray(xn[:, bs, :]).reshape(N, COLS).astype(s_np)
        im = {
            "xr": xr_c,
            "adjT": adjT_c,
            "xt_bf": xt_c.astype(BF16_NP),
            "deg": deg[None, rs].astype(BF16_NP),
            "b0a": b0a[None, :].astype(BF16_NP),
            "wx": Wx.astype(BF16_NP),
            "wa": Wa.astype(BF16_NP),
            "wo": Wo.astype(BF16_NP),
            "b12": b12,
        }
        if variant in ("dr2", "dr3", "dr4"):
            im["bo"] = bo.reshape(C, 1).copy()
        else:
            im["xtbo"] = np.ascontiguousarray(xt_c + bo[:, None])
        in_maps.append(im)
    return in_maps


def assemble_output(results):
    out = np.empty((B, N, C), np.float32)
    for c in range(NCORES):
        g = c // BG
        p = c % BG
        r = results[c]["out"]                      # [C, (b, node)] f32
        out[p * BP:(p + 1) * BP, g * NS2:(g + 1) * NS2, :] = (
            r.reshape(C, BP, NS2).transpose(1, 2, 0))
    return out


_NC_CACHE = []


def kernel(x, edge_index, W0, b0, W1, b1, W2, b2, Wo, bo):
    in_maps = host_prep(x, edge_index, W0, b0, W1, b1, W2, b2, Wo, bo)
    if not _NC_CACHE:
        _NC_CACHE.append(build_bass())
    nc = _NC_CACHE[0]
    res = run_bass_kernel_spmd(nc, in_maps, list(range(NCORES)))
    return assemble_output(res.results)



# revision 15
# speedup vs baseline: 1.0183x; 1.0183x over previous
"""GCNConv-style GNN layer on 8 Trainium2 NeuronCores (Bass/Tile).

Reference computation (B=8, N=4096, C=128, E=131072):
    adj  = symmetric 0/1 adjacency from edge_index, zero diagonal
    h    = x @ W0 + b0
    agg  = adj @ h            (per batch)
    out  = (cat[x, agg] @ W1 + b1) @ W2 + b2
    out  = gelu(out) @ Wo + bo
    ret  = x + out

Algebraic refactor (all linear maps before the single GELU compose; fold
them on the host at O(C^2) cost):
    W12  = W1 @ W2                  [2C, C]
    Wx   = W12[:C]                  x-path weight
    Wa   = W0 @ W12[C:]             agg-path weight applied to s = adj @ x
    b0a  = b0 @ W12[C:]
    b12  = b1 @ W2 + b2
    pre  = x @ Wx + (adj @ x) @ Wa + deg (x) b0a + b12
    ret  = x + gelu(pre) @ Wo + bo
where deg = adj.sum(1).

Sharding: 2 node-groups x 4 batch-groups (each core: 2048 nodes, 2
batches).  This keeps the per-core adjacency slice at 8MB fp8 (a bf16
slice starves the PE behind the ~240GB/s per-stream DMA rate) and gives
each stationary operand of the s = adj @ x contraction 4 moving matmuls
of reuse.

The graded variant is "dr2": the s-stage runs in fp8 e4m3 with
perf_mode=DoubleRow, packing two 128-row k-chunks per matmul.  Measured
on this hardware a DR matmul streams at the bf16 element rate (no fp8
double-pump through the walrus-mandated slab-split ifmap AP), so the
win over bf16 is halved instruction count (each matmul pays its serial
~213ns weight load) and halved adjacency DMA.  The fused MLP stays bf16
with fp32 PSUM accumulation; the residual uses bf16 x plus an exact bo
bias on the ACT engine (adds ~2^-9|x| error, net 1.22e-2 vs the 2e-2
gate -- verified against the reference on CPU and on hardware).  Unused
per-matmul PE semaphore increments are stripped in the single-shot
build.  Other variants ("bf16", "fp8", "dr", "dr_pa", "swi") are kept
for A/B experiments driven by test.py.
"""

import numpy as np
import ml_dtypes

import bass_rust
import concourse.bass as bass
import concourse.mybir as mybir
import concourse.tile as tile
from concourse.bass_utils import run_bass_kernel_spmd

B, N, C, E = 8, 4096, 128, 131072
NCORES = 8
NG = 2                 # node groups
BG = 4                 # batch groups
NS2 = N // NG          # 2048 nodes per core
BP = B // BG           # 2 batches per core
COLS = BP * C          # 256 xr columns (b-major, c-minor)
RCOLS = BP * NS2       # 4096 output columns (b-major, node-minor)
KC = N // 128          # 32 k-chunks over the contraction dim
KP = KC // 2           # 16 DoubleRow k-pairs
NJ = NS2 // 512        # 4 node chunks of 512 per core

F32 = mybir.dt.float32
BF16 = mybir.dt.bfloat16
F8 = mybir.dt.float8e4
BF16_NP = ml_dtypes.bfloat16
F8_NP = ml_dtypes.float8_e4m3

VARIANT = "dr2"        # "bf16" | "fp8" | "dr" | "dr2" | "dr_pa" | "swi" | "dr5" | "dr5b"
WARM_MM = 28           # HAM warmup matmuls for the dr5 variants


def _split_multiwaits(nc, max_waits=1):
    """Walrus (CoreV3) refuses instructions with more than one sync wait.
    Tile's tail drain can carry several; hoist the extras onto preceding
    single-wait EventSemaphore instructions on the same engine."""
    for blk in nc.m.functions[0].blocks:
        new_list = []
        for ins in blk.instructions:
            si = ins.sync_info
            if si is not None and si.on_wait and len(si.on_wait) > max_waits:
                waits = list(si.on_wait)
                extra, keep = waits[:-max_waits], waits[-max_waits:]
                for i, w in enumerate(extra):
                    ev = mybir.InstEventSemaphore(
                        name=f"{ins.name}_wsplit{i}",
                        engine=ins.engine,
                        ins=[],
                        outs=[],
                        sync_info=bass_rust.SyncInfo(on_wait=[w], on_update=[]),
                    )
                    new_list.append(ev)
                si.on_wait = keep
            new_list.append(ins)
        blk.instructions[:] = new_list


def _dedup_ldweights(nc):
    """Bass legalization emits one InstLdweights per InstMatmult even when
    consecutive matmuls share the identical stationary operand.  Drop
    loads whose weights AP matches the PE array's current contents.  Only
    loads with no sync waits/updates are dropped (the first load of each
    reuse group carries the DMA wait and is kept)."""

    def sig(ins):
        w = ins.ins[0]
        return (
            w.offset,
            tuple(tuple(p) for p in w.ap),
            str(w.dtype),
            getattr(w, "memref", None),
            getattr(w, "memsetref", None),
            str(ins.perf_mode),
            bool(ins.is_transpose),
        )

    for blk in nc.m.functions[0].blocks:
        last = None
        kept = []
        for ins in blk.instructions:
            tn = type(ins).__name__
            if tn == "InstLdweights":
                si = ins.sync_info
                clean = si is None or (not si.on_wait and not si.on_update)
                s = sig(ins)
                if clean and s == last:
                    continue
                last = s
            kept.append(ins)
        blk.instructions[:] = kept


def _fuse_ldweights(nc):
    """Remove the legalization-split InstLdweights and mark each
    InstMatmult self-loading, migrating the load's sync waits onto the
    matmul.  Required for walrus --enable-ldw-opt=true, which rejects
    explicit InstLdweights but then optimizes the weight loads itself."""
    for blk in nc.m.functions[0].blocks:
        kept = []
        pending = []
        for ins in blk.instructions:
            tn = type(ins).__name__
            if tn == "InstLdweights":
                si = ins.sync_info
                if si is not None:
                    pending.extend(si.on_wait or [])
                    assert not si.on_update, "ldweights with updates unsupported"
                continue
            if tn == "InstMatmult":
                ins.ldweights = True
                if pending:
                    si = ins.sync_info
                    if si is None:
                        ins.sync_info = bass_rust.SyncInfo(
                            on_wait=list(pending), on_update=[])
                    else:
                        si.on_wait = list(pending) + list(si.on_wait or [])
                    pending = []
            kept.append(ins)
        assert not pending, "dangling ldweights waits"
        blk.instructions[:] = kept


def _strip_pe_semupds(nc, relocate=False):
    """Every InstMatmult increments the PE semaphore, but only ~25 of the
    160 values are ever awaited.  Strip the unneeded increments (engine
    EVT_SEM writes cost serial issue time) and rewrite every wait on the
    PE semaphore (in all blocks) to the new, sparser counting.
    relocate=True re-emits the stripped increments as InstEventSemaphore
    ticks after the last matmul, preserving the per-iteration total that
    the Tile hardware-loop boundary requires (mandatory for niter>1)."""
    blocks = nc.m.functions[0].blocks
    pe_incs = []  # (block, instruction, update) in program order
    for blk in blocks:
        for ins in blk.instructions:
            si = ins.sync_info
            if si is None:
                continue
            for u in (si.on_update or []):
                if u.ant_name.startswith("PE_"):
                    pe_incs.append((blk, ins, u))
    if len(pe_incs) < 8:
        return
    total = len(pe_incs)  # walrus enforces update_value==1
    needed_vals = set()
    for blk in blocks:
        for ins in blk.instructions:
            si = ins.sync_info
            if si is None:
                continue
            for w in (si.on_wait or []):
                if w.ant_name.startswith("PE_") and w.wait_value is not None:
                    needed_vals.add(w.wait_value)
    needed_idx = set()
    for v in needed_vals:
        assert v <= total, f"PE wait {v} beyond total {total}"
        needed_idx.add(v - 1)  # inc #v (1-based) satisfies value v
    needed_idx.add(total - 1)  # keep the final one
    new_cum = []
    kept = 0
    for i in range(total):
        if i in needed_idx:
            kept += 1
        new_cum.append(kept)
    n_stripped = total - kept
    if not relocate:
        val_map = {v: new_cum[v - 1] for v in needed_vals}
        for blk in blocks:
            for ins in blk.instructions:
                si = ins.sync_info
                if si is None:
                    continue
                for w in (si.on_wait or []):
                    if w.ant_name.startswith("PE_") and w.wait_value is not None:
                        w.wait_value = val_map[w.wait_value]
    else:
        # kept incs count first; the relocated tail ticks bring the total
        # back to `total`, so only intermediate waits need remapping
        val_map = {v: new_cum[v - 1] for v in needed_vals}
        for blk in blocks:
            for ins in blk.instructions:
                si = ins.sync_info
                if si is None:
                    continue
                for w in (si.on_wait or []):
                    if (w.ant_name.startswith("PE_")
                            and w.wait_value is not None
                            and w.wait_value < total):
                        w.wait_value = val_map[w.wait_value]
    for i, (blk, ins, u) in enumerate(pe_incs):
        if i not in needed_idx:
            ins.sync_info.on_update = [x for x in ins.sync_info.on_update if x is not u]
    if relocate and n_stripped:
        lb, last_ins, lu = pe_incs[-1]
        pos = lb.instructions.index(last_ins) + 1
        ticks = []
        for i in range(n_stripped):
            ticks.append(mybir.InstEventSemaphore(
                name=f"{last_ins.name}_semtick{i}",
                engine=last_ins.engine,
                ins=[],
                outs=[],
                sync_info=bass_rust.SyncInfo(
                    on_wait=[],
                    on_update=[bass_rust.SyncUpdate(
                        sync_type=lu.sync_type,
                        id=lu.id,
                        ant_name=lu.ant_name,
                        update_mode=lu.update_mode,
                        update_value=1,
                        update_reg=None,
                    )],
                ),
            ))
        lb.instructions[pos:pos] = ticks


_LDWOPT_PATCHED = []


def _patch_ldwopt():
    """Flip walrus --enable-ldw-opt to true for subsequent compiles."""
    if _LDWOPT_PATCHED:
        return
    import concourse.bass_utils as _bu

    _orig = _bu.run_command

    def patched(cmd, **kw):
        cmd = [
            c.replace("--enable-ldw-opt=false", "--enable-ldw-opt=true")
            if isinstance(c, str) else c
            for c in cmd
        ]
        return _orig(cmd, **kw)

    _bu.run_command = patched
    _LDWOPT_PATCHED.append(True)


def build_bass(niter=1, stage="full", variant=None, dedup=False, ldwopt=False,
               strip=True):
    """Build the SPMD program.  niter>1 wraps the body in a Tile For_i
    loop -- used only for hardware timing (amortizes axon dispatch
    overhead); the graded kernel uses niter=1.
    stage: "full" | "s_only" (timing ablation)."""
    variant = variant or VARIANT
    sdt = BF16 if variant == "bf16" else F8
    nc = bass.Bass()

    if variant in ("dr5", "dr5b", "dr6", "dr6b"):
        # partition-major everything: xr as dr2; adjacency [128, KC*NS2]
        # with adjpm[p, k*NS2+j] = adj[128k+p, j] so ~1MB contiguous
        # per-partition slices stream at full HBM rate.  For dr6 the xr
        # payload is y = x@Wa + 1*b0a (the whole agg path folded on host:
        # adj@y = (adj@x)@Wa + deg*b0a), so wa/deg/b0a/rank-1 disappear.
        xr_d = nc.dram_tensor("xr", [128, KC * COLS], F8, kind="ExternalInput")
        adjT_d = nc.dram_tensor("adjT", [128, KC * NS2], F8, kind="ExternalInput")
    elif variant in ("dr2", "dr3", "dr4"):
        # partition-major xr (one contiguous DMA), no f32 residual input
        xr_d = nc.dram_tensor("xr", [128, KC * COLS], F8, kind="ExternalInput")
        if variant == "dr4":
            # partition-major pair-merged adjacency: 16 DMAs, 4KB lines
            adjT_d = nc.dram_tensor("adjT", [128, KC * NS2], F8, kind="ExternalInput")
        else:
            adjT_d = nc.dram_tensor("adjT", [N, NS2], F8, kind="ExternalInput")
    elif variant == "dr_pa":
        # pair-adjacent fp8 layout: dram matches SBUF exactly, partition-major
        xr_d = nc.dram_tensor("xr", [128, KP * BP * 128 * 2], F8, kind="ExternalInput")
        adjT_d = nc.dram_tensor("adjT", [128, KP * NS2 * 2], F8, kind="ExternalInput")
    elif variant == "swi":
        # sw-interleaved stationary (contiguous per-(t,b) 256-col weight),
        # adjacency chunk layout as in "dr"
        xr_d = nc.dram_tensor("xr", [128, KP * BP * 256], F8, kind="ExternalInput")
        adjT_d = nc.dram_tensor("adjT", [N, NS2], F8, kind="ExternalInput")
    else:
        xr_d = nc.dram_tensor("xr", [N, COLS], sdt, kind="ExternalInput")
        adjT_d = nc.dram_tensor("adjT", [N, NS2], sdt, kind="ExternalInput")
    xt_bf_d = nc.dram_tensor("xt_bf", [C, RCOLS], BF16, kind="ExternalInput")
    if variant in ("dr2", "dr3", "dr4", "dr5", "dr5b"):
        bo_d = nc.dram_tensor("bo", [C, 1], F32, kind="ExternalInput")
    else:
        xtbo_d = nc.dram_tensor("xtbo", [C, RCOLS], F32, kind="ExternalInput")
    deg_d = nc.dram_tensor("deg", [1, NS2], BF16, kind="ExternalInput")
    b0a_d = nc.dram_tensor("b0a", [1, C], BF16, kind="ExternalInput")
    wx_d = nc.dram_tensor("wx", [C, C], BF16, kind="ExternalInput")
    wa_d = nc.dram_tensor("wa", [C, C], BF16, kind="ExternalInput")
    wo_d = nc.dram_tensor("wo", [C, C], BF16, kind="ExternalInput")
    b12_d = nc.dram_tensor("b12", [C, 1], F32, kind="ExternalInput")
    out_dt = BF16 if variant == "dr5b" else F32
    out_d = nc.dram_tensor("out", [C, RCOLS], out_dt, kind="ExternalOutput")

    with tile.TileContext(nc) as tc:
        with (
            tc.tile_pool(name="const", bufs=1) as const,
            tc.tile_pool(name="big", bufs=1) as big,
        ):

            def body(_iv=0):
                # ---- resident inputs -------------------------------------
                v5 = variant in ("dr5", "dr5b")
                if variant == "dr_pa":
                    xr_sb = big.tile([128, KP, BP * 128, 2], F8)
                    adjT_sb = big.tile([128, KP, NS2, 2], F8)
                elif variant == "swi":
                    xr_sb = big.tile([128, KP, BP, 256], F8)
                    adjT_sb = big.tile([128, KC, NS2], F8)
                else:
                    xr_sb = big.tile([128, KC, COLS], sdt)
                    adjT_sb = big.tile([128, KC, NS2], sdt)
                xt_bf_sb = big.tile([C, RCOLS], BF16)
                if variant in ("dr2", "dr3", "dr4", "dr5", "dr5b"):
                    bo_sb = const.tile([C, 1], F32)
                else:
                    xtbo_sb = big.tile([C, RCOLS], F32)
                wx_sb = const.tile([C, C], BF16)
                wa_sb = const.tile([C, C], BF16)
                wo_sb = const.tile([C, C], BF16)
                b12_sb = const.tile([C, 1], F32)
                if variant == "dr3":
                    # deg/b0a replicated at partitions 0/32/64/96 so the 8
                    # rank-1 matmuls can pack 4-concurrent into distinct PE
                    # row groups
                    deg_sb = const.tile([128, NS2], BF16)
                    b0a_sb = const.tile([128, C], BF16)
                else:
                    deg_sb = const.tile([1, NS2], BF16)
                    b0a_sb = const.tile([1, C], BF16)
                if v5:
                    # consts first on both queues (gate warmup + MLP)
                    nc.sync.dma_start(out=wx_sb[:], in_=wx_d[:])
                    nc.scalar.dma_start(out=wa_sb[:], in_=wa_d[:])
                    nc.scalar.dma_start(out=wo_sb[:], in_=wo_d[:])
                    nc.scalar.dma_start(out=b12_sb[:], in_=b12_d[:])
                    nc.scalar.dma_start(out=deg_sb[:], in_=deg_d[:])
                    nc.scalar.dma_start(out=b0a_sb[:], in_=b0a_d[:])
                    nc.scalar.dma_start(out=bo_sb[:], in_=bo_d[:])
                    # xr split x4 on the scalar queue: stationary chunks
                    # arrive in t-order alongside the adjacency stream
                    for q in range(4):
                        nc.scalar.dma_start(
                            out=xr_sb[:, 8 * q:8 * (q + 1), :],
                            in_=xr_d[:, 8 * q * COLS:8 * (q + 1) * COLS])
                    # adjacency on the sync queue: 2 x 512KB head chunks
                    # (PE starts after ~1 t-pair), then 7 x 1MB
                    plan = [2, 2] + [4] * 7
                    k0 = 0
                    for nk in plan:
                        nc.sync.dma_start(
                            out=adjT_sb[:, k0:k0 + nk, :],
                            in_=adjT_d[:, k0 * NS2:(k0 + nk) * NS2])
                        k0 += nk
                    # xt_bf last (needed only at the MLP stage)
                    nc.scalar.dma_start(out=xt_bf_sb[:], in_=xt_bf_d[:])
                elif variant == "dr_pa":
                    for t in range(KP):
                        nc.sync.dma_start(out=adjT_sb[:, t, :, :],
                                          in_=adjT_d[:, t * NS2 * 2:(t + 1) * NS2 * 2])
                        nc.sync.dma_start(out=xr_sb[:, t, :, :],
                                          in_=xr_d[:, t * BP * 256:(t + 1) * BP * 256])
                elif variant == "swi":
                    for k in range(KC):
                        nc.sync.dma_start(out=adjT_sb[:, k, :], in_=adjT_d[k * 128:(k + 1) * 128, :])
                    nc.sync.dma_start(out=xr_sb[:, :, :, :], in_=xr_d[:, :])
                elif variant == "dr2":
                    nc.sync.dma_start(out=xr_sb[:, :, :], in_=xr_d[:, :])
                    for k in range(KC):
                        nc.sync.dma_start(out=adjT_sb[:, k, :], in_=adjT_d[k * 128:(k + 1) * 128, :])
                elif variant == "dr4":
                    nc.sync.dma_start(out=xr_sb[:, :, :], in_=xr_d[:, :])
                    for t in range(KP):
                        nc.sync.dma_start(out=adjT_sb[:, 2 * t:2 * t + 2, :],
                                          in_=adjT_d[:, t * 2 * NS2:(t + 1) * 2 * NS2])
                elif variant == "dr3":
                    # first 4 chunks land fast so the PE starts sooner; the
                    # rest follows as one big transfer
                    nc.sync.dma_start(out=xr_sb[:, 0:4, :], in_=xr_d[:, 0:4 * COLS])
                    nc.sync.dma_start(out=xr_sb[:, 4:KC, :], in_=xr_d[:, 4 * COLS:])
                    for k in range(KC):
                        nc.sync.dma_start(out=adjT_sb[:, k, :], in_=adjT_d[k * 128:(k + 1) * 128, :])
                else:
                    for k in range(KC):
                        nc.sync.dma_start(out=adjT_sb[:, k, :], in_=adjT_d[k * 128:(k + 1) * 128, :])
                        nc.sync.dma_start(out=xr_sb[:, k, :], in_=xr_d[k * 128:(k + 1) * 128, :])
                if not v5:
                    nc.sync.dma_start(out=wx_sb[:], in_=wx_d[:])
                    nc.sync.dma_start(out=wa_sb[:], in_=wa_d[:])
                    nc.sync.dma_start(out=wo_sb[:], in_=wo_d[:])
                    nc.sync.dma_start(out=b12_sb[:], in_=b12_d[:])
                    if variant == "dr3":
                        nc.sync.dma_start(out=deg_sb[0:1, :], in_=deg_d[:])
                        nc.sync.dma_start(out=b0a_sb[0:1, :], in_=b0a_d[:])
                        for q in (32, 64, 96):
                            nc.vector.tensor_copy(out=deg_sb[q:q + 1, :], in_=deg_sb[0:1, :])
                            nc.vector.tensor_copy(out=b0a_sb[q:q + 1, :], in_=b0a_sb[0:1, :])
                    else:
                        nc.sync.dma_start(out=deg_sb[:], in_=deg_d[:])
                        nc.sync.dma_start(out=b0a_sb[:], in_=b0a_d[:])
                    nc.sync.dma_start(out=xt_bf_sb[:], in_=xt_bf_d[:])
                    if variant in ("dr2", "dr3", "dr4"):
                        nc.sync.dma_start(out=bo_sb[:], in_=bo_d[:])
                    else:
                        nc.sync.dma_start(out=xtbo_sb[:], in_=xtbo_d[:])

                # ---- s = adj @ x, transposed: accumulator (b, j) holds
                # sT[c, node-chunk j] for local batch b.  Stationary =
                # xr k-chunk for batch b, reused across the 4 node-chunk
                # moving operands.  k outer keeps the PE chasing the DMA
                # stream from the first chunk. ----
                sT_sb = big.tile([C, RCOLS], BF16)
                gelu_sb = big.tile([C, RCOLS], BF16)
                res_sb = big.tile([C, RCOLS], out_dt)
                chunks = [(b, j) for b in range(BP) for j in range(NJ)]

                with tc.tile_pool(name="psum", bufs=8, space="PSUM") as psum:
                    if v5 and WARM_MM:
                        # junk matmuls gated only on the (tiny, first-issued)
                        # wx DMA: spin the PE past the ~3.4us HAM window at
                        # 1.2GHz so the real s-stage starts at 2.4GHz
                        warm = psum.tile([128, 512], F32, tag="ps", name="warm")
                        for _ in range(WARM_MM):
                            nc.tensor.matmul(warm[:, 0:C], wx_sb[:], wx_sb[:],
                                             start=True, stop=True)
                    ps = {
                        (b, j): psum.tile([128, 512], F32, tag="ps", name=f"sT_{b}_{j}")
                        for (b, j) in chunks
                    }
                    if variant in ("dr", "dr2", "dr3", "dr4", "dr5", "dr5b"):
                        for t in range(KP):
                            for b in range(BP):
                                lhsT = xr_sb[:, 2 * t:2 * t + 2, b * 128:(b + 1) * 128]
                                for j in range(NJ):
                                    nc.tensor.matmul(
                                        ps[(b, j)],
                                        lhsT,
                                        adjT_sb[:, 2 * t:2 * t + 2, j * 512:(j + 1) * 512],
                                        start=(t == 0),
                                        stop=(t == KP - 1),
                                        perf_mode=mybir.MatmulPerfMode.DoubleRow,
                                    )
                    elif variant == "dr_pa":
                        for t in range(KP):
                            for b in range(BP):
                                lhsT = xr_sb[:, t, b * 128:(b + 1) * 128, :]
                                for j in range(NJ):
                                    nc.tensor.matmul(
                                        ps[(b, j)],
                                        lhsT,
                                        adjT_sb[:, t, j * 512:(j + 1) * 512, :],
                                        start=(t == 0),
                                        stop=(t == KP - 1),
                                        perf_mode=mybir.MatmulPerfMode.DoubleRow,
                                    )
                    elif variant == "swi":
                        for t in range(KP):
                            for b in range(BP):
                                lhsT = xr_sb[:, t, b, :]
                                for j in range(NJ):
                                    nc.tensor.matmul(
                                        ps[(b, j)],
                                        lhsT,
                                        adjT_sb[:, 2 * t:2 * t + 2, j * 512:(j + 1) * 512],
                                        start=(t == 0),
                                        stop=(t == KP - 1),
                                        perf_mode=mybir.MatmulPerfMode.DoubleRowSwInterleave,
                                    )
                    else:
                        for t in range(KC):
                            for b in range(BP):
                                lhsT = xr_sb[:, t, b * 128:(b + 1) * 128]
                                for j in range(NJ):
                                    nc.tensor.matmul(
                                        ps[(b, j)],
                                        lhsT,
                                        adjT_sb[:, t, j * 512:(j + 1) * 512],
                                        start=(t == 0),
                                        stop=(t == KC - 1),
                                    )

                    # evacs on DVE begin as soon as each bank's stop MM
                    # retires (bank (0,0) frees 7 MMs before s ends)
                    for (b, j) in chunks:
                        cs = slice(b * NS2 + j * 512, b * NS2 + (j + 1) * 512)
                        nc.vector.tensor_copy(out=sT_sb[:, cs], in_=ps[(b, j)])

                    if stage != "full":
                        nc.sync.dma_start(
                            out=out_d[:, 0:RCOLS // 2],
                            in_=sT_sb.bitcast(F32)[:, 0:RCOLS // 2],
                        )
                        return

                    # ---- fused MLP, one 8-wide wave: each weight is
                    # loaded once and streamed over all 8 chunks.  pre
                    # reuses the s banks (freed by the DVE evacs), po
                    # reuses them again (freed by the ACT gelu reads).
                    def cslice(b, j):
                        return slice(b * NS2 + j * 512, b * NS2 + (j + 1) * 512)

                    pre = {
                        (b, j): psum.tile([128, 512], F32, tag="ps", name=f"pre_{b}_{j}")
                        for (b, j) in chunks
                    }
                    for (b, j) in chunks:
                        nc.tensor.matmul(pre[(b, j)], wx_sb[:], xt_bf_sb[:, cslice(b, j)],
                                         start=True, stop=False)
                    for (b, j) in chunks:
                        nc.tensor.matmul(pre[(b, j)], wa_sb[:], sT_sb[:, cslice(b, j)],
                                         start=False, stop=False)
                    if variant == "dr3":
                        # 4-way row-group packing: K=1 matmuls at partition
                        # bases 0/32/64/96 run concurrently in the PE array
                        for i, (b, j) in enumerate(chunks):
                            q = (i % 4) * 32
                            nc.tensor.matmul(
                                pre[(b, j)],
                                b0a_sb[q:q + 1, :],
                                deg_sb[q:q + 1, j * 512:(j + 1) * 512],
                                start=False, stop=True,
                                tile_position=(q, 0),
                            )
                    else:
                        for (b, j) in chunks:
                            nc.tensor.matmul(pre[(b, j)], b0a_sb[:], deg_sb[:, j * 512:(j + 1) * 512],
                                             start=False, stop=True)
                    for (b, j) in chunks:
                        nc.scalar.activation(
                            out=gelu_sb[:, cslice(b, j)], in_=pre[(b, j)],
                            func=mybir.ActivationFunctionType.Gelu,
                            bias=b12_sb[:, 0:1], scale=1.0,
                        )
                    po = {
                        (b, j): psum.tile([128, 512], F32, tag="ps", name=f"out_{b}_{j}")
                        for (b, j) in chunks
                    }
                    for (b, j) in chunks:
                        nc.tensor.matmul(po[(b, j)], wo_sb[:], gelu_sb[:, cslice(b, j)],
                                         start=True, stop=True)
                    for (b, j) in chunks:
                        cs = cslice(b, j)
                        if variant in ("dr2", "dr3", "dr4", "dr5", "dr5b"):
                            # ACT evacuates po with the bo bias; DVE adds the
                            # bf16 residual (error ~2^-9 |x|, well in budget)
                            nc.scalar.activation(
                                out=res_sb[:, cs], in_=po[(b, j)],
                                func=mybir.ActivationFunctionType.Identity,
                                bias=bo_sb[:, 0:1], scale=1.0,
                            )
                            nc.vector.tensor_add(out=res_sb[:, cs], in0=res_sb[:, cs],
                                                 in1=xt_bf_sb[:, cs])
                        else:
                            nc.vector.tensor_add(out=res_sb[:, cs], in0=po[(b, j)],
                                                 in1=xtbo_sb[:, cs])
                        nc.sync.dma_start(out=out_d[:, cs], in_=res_sb[:, cs])

            if niter == 1:
                body()
            else:
                with tc.For_i(0, niter, 1, hint_engines=(mybir.EngineType.PE,)):
                    body()

    if ldwopt:
        _fuse_ldweights(nc)
        _patch_ldwopt()
    elif dedup:
        _dedup_ldweights(nc)
    if variant in ("dr2", "dr3", "dr4", "dr5", "dr5b") and strip and niter == 1:
        # the sparser counting is incompatible with Tile's For_i iteration
        # accounting (hangs for niter>1); the graded single-shot build is safe
        _strip_pe_semupds(nc)
    _split_multiwaits(nc)
    return nc


def host_prep(x, edge_index, W0, b0, W1, b1, W2, b2, Wo, bo, variant=None):
    """Fold weights, build the dense adjacency, lay out per-core inputs."""
    variant = variant or VARIANT
    x = np.asarray(x, np.float32)
    ei = np.asarray(edge_index, np.int64)
    W0, b0, W1, b1, W2, b2, Wo, bo = (
        np.asarray(a, np.float32) for a in (W0, b0, W1, b1, W2, b2, Wo, bo)
    )

    # dense symmetric adjacency with set-semantics dedup, zero diagonal
    k1 = ei[0] * N + ei[1]
    k2 = ei[1] * N + ei[0]
    keys = np.unique(np.concatenate([k1, k2]))
    rows = keys // N
    cols = keys % N
    off_diag = rows != cols
    keys, rows = keys[off_diag], rows[off_diag]
    if variant == "bf16":
        adj = np.zeros(N * N, np.uint16)
        adj[keys] = 0x3F80  # bf16 1.0 bit pattern
        adj = adj.reshape(N, N).view(BF16_NP)
        s_np = BF16_NP
    else:
        adj = np.zeros(N * N, np.uint8)
        adj[keys] = 0x38  # fp8 e4m3 1.0 bit pattern
        adj = adj.reshape(N, N).view(F8_NP)
        s_np = F8_NP
    deg = np.bincount(rows, minlength=N).astype(np.float32)

    # folded weights
    W12 = W1 @ W2                      # [2C, C]
    Wx = W12[:C]
    W12a = W12[C:]
    Wa = W0 @ W12a
    b0a = b0 @ W12a                    # [C]
    b12 = (b1 @ W2 + b2).reshape(C, 1)

    xn = x.transpose(1, 0, 2)                                  # [N, B, C]
    xt = x.transpose(2, 0, 1)                                  # [C, B, N] f32

    in_maps = []
    for c in range(NCORES):
        g = c // BG                    # node group
        p = c % BG                     # batch pair
        rs = slice(g * NS2, (g + 1) * NS2)
        bs = slice(p * BP, (p + 1) * BP)
        xt_c = np.ascontiguousarray(xt[:, bs, rs]).reshape(C, RCOLS)
        if variant == "dr_pa":
            # pair-adjacent fp8: byte o of the 16-bit fetch at (p, col) is
            # contraction row 256t + 128o + p
            a = np.ascontiguousarray(adj[:, rs]).reshape(KP, 2, 128, NS2)
            adjT_c = np.ascontiguousarray(a.transpose(2, 0, 3, 1)).reshape(128, -1)
            xv = xn[:, bs, :].reshape(KP, 2, 128, BP, C).astype(s_np)
            xr_c = np.ascontiguousarray(xv.transpose(2, 0, 3, 4, 1)).reshape(128, -1)
        elif variant == "swi":
            # sw-interleaved stationary: per (t, b) the 256 weight columns are
            # [A127, B127, A126, B126, ...] (slabs interleaved, columns
            # reversed); A = k-chunk 2t, B = k-chunk 2t+1
            adjT_c = np.ascontiguousarray(adj[:, rs])
            xv = xn[:, bs, :].reshape(KP, 2, 128, BP, C).astype(s_np)
            rev = xv[:, :, :, :, ::-1]
            xr_c = np.ascontiguousarray(rev.transpose(2, 0, 3, 4, 1)).reshape(128, -1)
        elif variant == "dr4":
            a = np.ascontiguousarray(adj[:, rs]).reshape(KP, 2, 128, NS2)
            adjT_c = np.ascontiguousarray(a.transpose(2, 0, 1, 3)).reshape(128, -1)
            xv = xn[:, bs, :].reshape(KC, 128, COLS).astype(s_np)
            xr_c = np.ascontiguousarray(xv.transpose(1, 0, 2)).reshape(128, -1)
        elif variant in ("dr5", "dr5b"):
            # partition-major adjacency: row 128k+p of adj[:, rs] lands at
            # partition p, chunk k -- contiguous ~1MB per-partition slices
            a = np.ascontiguousarray(adj[:, rs]).reshape(KC, 128, NS2)
            adjT_c = np.ascontiguousarray(a.transpose(1, 0, 2)).reshape(128, -1)
            xv = xn[:, bs, :].reshape(KC, 128, COLS).astype(s_np)
            xr_c = np.ascontiguousarray(xv.transpose(1, 0, 2)).reshape(128, -1)
        elif variant in ("dr2", "dr3"):
            adjT_c = np.ascontiguousarray(adj[:, rs])
            xv = xn[:, bs, :].reshape(KC, 128, COLS).astype(s_np)
            xr_c = np.ascontiguousarray(xv.transpose(1, 0, 2)).reshape(128, -1)
        else:
            adjT_c = np.ascontiguousarray(adj[:, rs])
            xr_c = np.ascontiguousarray(xn[:, bs, :]).reshape(N, COLS).astype(s_np)
        im = {
            "xr": xr_c,
            "adjT": adjT_c,
            "xt_bf": xt_c.astype(BF16_NP),
            "deg": deg[None, rs].astype(BF16_NP),
            "b0a": b0a[None, :].astype(BF16_NP),
            "wx": Wx.astype(BF16_NP),
            "wa": Wa.astype(BF16_NP),
            "wo": Wo.astype(BF16_NP),
            "b12": b12,
        }
        if variant in ("dr2", "dr3", "dr4", "dr5", "dr5b"):
            im["bo"] = bo.reshape(C, 1).copy()
        else:
            im["xtbo"] = np.ascontiguousarray(xt_c + bo[:, None])
        in_maps.append(im)
    return in_maps


def assemble_output(results):
    out = np.empty((B, N, C), np.float32)
    for c in range(NCORES):
        g = c // BG
        p = c % BG
        r = results[c]["out"]                      # [C, (b, node)] f32
        out[p * BP:(p + 1) * BP, g * NS2:(g + 1) * NS2, :] = (
            r.reshape(C, BP, NS2).transpose(1, 2, 0))
    return out


_NC_CACHE = []


def kernel(x, edge_index, W0, b0, W1, b1, W2, b2, Wo, bo):
    in_maps = host_prep(x, edge_index, W0, b0, W1, b1, W2, b2, Wo, bo)
    if not _NC_CACHE:
        _NC_CACHE.append(build_bass())
    nc = _NC_CACHE[0]
    res = run_bass_kernel_spmd(nc, in_maps, list(range(NCORES)))
    return assemble_output(res.results)

